# revision 1
# baseline (speedup 1.0000x reference)
"""Trainium2 Bass kernel for LocalXLAttention (chunk-summed variant).

Math: the reference einsum sums over the chunk index z, so every query
attends to the same three [w, dh] K/V matrices built from chunk sums:
  K_prev = S_k - k_chunk[C-1], K_cur = S_k, K_next = S_k - k_chunk[0]
(and identically for V), where S_k = sum_c k_chunk[c].  The computation
collapses to, per sequence position l and head h:
  attn[l,h,:]  = qp[l,h,:] @ KbigT          (KbigT: [dh, 3w])
  probs        = softmax(attn, axis=-1)
  ctx[l,h,:]   = probs[l,h,:] @ Vbig        (Vbig:  [3w, dh])
  out          = ctx.reshape(L, dm) @ Wc

Sharding: L=4096 is split 512 rows per core across 8 NeuronCores
(data-parallel over the sequence; no collectives).  Each core redundantly
computes the tiny chunk-summed K/V from the full kv input.

The attention pipeline runs fully transposed ([j, l] / [he, l] layouts) so
no on-device transposes of activations are needed; probs normalization is
deferred to the context (an extra all-ones column of Vbig accumulates the
softmax denominator for free).

Matmuls run in float32r (TF32-class PE mode, 1 cycle/row vs 4 for fp32).
"""

import sys
for _p in ('/opt/pypackages', '/opt/trn_rl_repo'):
    if _p not in sys.path:
        sys.path.insert(0, _p)

import numpy as np

import concourse.bass as bass
import concourse.bacc as bacc
import concourse.tile as tile
from concourse import mybir
from concourse.bass_utils import run_bass_kernel_spmd
from concourse.masks import make_identity

F32 = mybir.dt.float32
F32R = mybir.dt.float32r
AF = mybir.ActivationFunctionType

N_CORES = 8
L = 4096          # full sequence
LS = L // N_CORES # 512 rows per core
DM = 1024
NH = 16
DH = 64
W = 512           # chunk width
C = L // W        # 8 chunks
J3 = 3 * W        # 1536 softmax width
NJ = J3 // 128    # 12 j-chunks
DMT = DM // 128   # 8 dm-chunks


def build_nc():
    nc = bacc.Bacc(None, target_bir_lowering=False)

    qT = nc.dram_tensor("qT", [DM, LS], F32R, kind="ExternalInput")
    kvT = nc.dram_tensor("kvT", [DM, L], F32R, kind="ExternalInput")
    Wq = nc.dram_tensor("Wq", [DM, DM], F32R, kind="ExternalInput")
    Wkv = nc.dram_tensor("Wkv", [DM, 2 * DH], F32R, kind="ExternalInput")
    Wc = nc.dram_tensor("Wc", [DM, DM], F32R, kind="ExternalInput")
    out = nc.dram_tensor("out", [LS, DM], F32, kind="ExternalOutput")

    with tile.TileContext(nc) as tc:
        with tc.tile_pool(name="weights", bufs=8) as wpool, \
             tc.tile_pool(name="small", bufs=1) as spool, \
             tc.tile_pool(name="qp", bufs=8) as qpool, \
             tc.tile_pool(name="qpt", bufs=4) as qptpool, \
             tc.tile_pool(name="stream", bufs=2) as stpool, \
             tc.tile_pool(name="kvsum", bufs=8) as kvspool, \
             tc.tile_pool(name="var", bufs=4) as varpool, \
             tc.tile_pool(name="probs", bufs=4) as ppool, \
             tc.tile_pool(name="misc", bufs=2) as mpool, \
             tc.tile_pool(name="dram", bufs=1, space="DRAM") as dpool, \
             tc.tile_pool(name="psacc", bufs=4, space="PSUM") as psacc, \
             tc.tile_pool(name="psmm", bufs=2, space="PSUM") as psmm:

            # ---------- load weights / q ----------
            wq_sb = []
            for d in range(DMT):
                t = wpool.tile([128, DM], F32R, tag="wq", name=f"wq{d}")
                nc.gpsimd.dma_start(out=t, in_=Wq[128 * d:128 * (d + 1), :])
                wq_sb.append(t)
            wkv_sb = []
            for d in range(DMT):
                t = wpool.tile([128, 2 * DH], F32R, tag="wkv", name=f"wkv{d}")
                nc.sync.dma_start(out=t, in_=Wkv[128 * d:128 * (d + 1), :])
                wkv_sb.append(t)
            qt_sb = []
            for d in range(DMT):
                t = qpool.tile([128, LS], F32R, tag="qt", name=f"qt{d}")
                nc.gpsimd.dma_start(out=t, in_=qT[128 * d:128 * (d + 1), :])
                qt_sb.append(t)

            ident = spool.tile([128, 128], F32, tag="ident")
            make_identity(nc, ident)

            # ---------- kv stream: chunk-sum (tree, in place) ----------
            # kvsum_sb[d][p, y] = sum_c kvT[128d+p, 512c + y]
            kvsum_sb = []
            k7p = psacc.tile([128, W], F32, tag="acc", name="k7p")
            v7p = psacc.tile([128, W], F32, tag="acc", name="v7p")
            for d in range(DMT):
                st = stpool.tile([128, L], F32R, tag="kvstream")
                nc.sync.dma_start(out=st[:, 0:L // 2],
                                  in_=kvT[128 * d:128 * (d + 1), 0:L // 2])
                nc.scalar.dma_start(out=st[:, L // 2:L],
                                    in_=kvT[128 * d:128 * (d + 1), L // 2:L])
                # chunk-7 columns [3584:4096] are only read (never written) by
                # the in-place tree below, so project k7/v7 straight from the
                # stream tile instead of reloading those columns later.
                nc.tensor.matmul(k7p[0:DH, :], wkv_sb[d][:, 0:DH],
                                 st[:, L - W:L], start=(d == 0),
                                 stop=(d == DMT - 1))
                nc.tensor.matmul(v7p[0:DH, :], wkv_sb[d][:, DH:2 * DH],
                                 st[:, L - W:L], start=(d == 0),
                                 stop=(d == DMT - 1))
                nc.vector.tensor_add(st[:, 0:2048], st[:, 0:2048], st[:, 2048:4096])
                nc.vector.tensor_add(st[:, 0:1024], st[:, 0:1024], st[:, 1024:2048])
                ks = kvspool.tile([128, W], F32R, tag="kvsum")
                nc.vector.tensor_add(ks, st[:, 0:512], st[:, 512:1024])
                kvsum_sb.append(ks)
            k7_sb = spool.tile([DH, W], F32, tag="k7")
            v7_sb = spool.tile([DH, W], F32, tag="v7")
            nc.vector.tensor_copy(k7_sb, k7p[0:DH, :])
            nc.vector.tensor_copy(v7_sb, v7p[0:DH, :])

            # ---------- QP_T = Wq.T @ q.T  (unscaled; 1/sqrt(dh) folded into exp) ----
            # qpt_sb[t][p, 512*half + l] = QP_T[hd = 128*(2t+half) + p, l]
            qpt_sb = []
            for t4 in range(4):
                ps = psmm.tile([128, 1024], F32, tag="mm")
                for half in range(2):
                    hd = 2 * t4 + half
                    for d in range(DMT):
                        nc.tensor.matmul(
                            ps[:, 512 * half:512 * (half + 1)],
                            wq_sb[d][:, 128 * hd:128 * (hd + 1)],
                            qt_sb[d],
                            start=(d == 0), stop=(d == DMT - 1))
                sb = qptpool.tile([128, 1024], F32R, tag="qpt")
                nc.vector.tensor_copy(sb, ps)
                qpt_sb.append(sb)

            # ---------- chunk-0 / chunk-7 K,V projections ----------
            # reload kvT columns for chunks 0 and 7 (the stream tiles are
            # mutated in place by the tree sum and rotate away).
            def project_variant(rhs_tiles, tag):
                """returns psum tiles (k [64,512], v [64,512]) accumulated
                over the 8 dm chunks of rhs_tiles (each [128, 512])."""
                kp = psacc.tile([128, W], F32, tag="acc")
                vp = psacc.tile([128, W], F32, tag="acc")
                for d in range(DMT):
                    nc.tensor.matmul(kp[0:DH, :], wkv_sb[d][:, 0:DH],
                                     rhs_tiles[d], start=(d == 0),
                                     stop=(d == DMT - 1))
                    nc.tensor.matmul(vp[0:DH, :], wkv_sb[d][:, DH:2 * DH],
                                     rhs_tiles[d], start=(d == 0),
                                     stop=(d == DMT - 1))
                return kp, vp

            kv0_sb = []
            for d in range(DMT):
                t0 = varpool.tile([128, W], F32R, tag="kv07", name=f"kv0_{d}")
                nc.scalar.dma_start(out=t0, in_=kvT[128 * d:128 * (d + 1), 0:W])
                kv0_sb.append(t0)

            k0_ps, v0_ps = project_variant(kv0_sb, "c0")
            # evacuate immediately so the psum slots can rotate
            k0_sb = spool.tile([DH, W], F32, tag="k0")
            v0_sb = spool.tile([DH, W], F32, tag="v0")
            nc.vector.tensor_copy(k0_sb, k0_ps[0:DH, :])
            nc.vector.tensor_copy(v0_sb, v0_ps[0:DH, :])

            ksum_ps, vsum_ps = project_variant(kvsum_sb, "sum")
            vsum_sb = spool.tile([DH, W], F32, tag="vsum")
            nc.vector.tensor_copy(vsum_sb, vsum_ps[0:DH, :])

            # ---------- KbigT [64, 1536] = [prev | cur | next] ----------
            # duplicated into partitions 64:128 so heads whose QP_T rows sit
            # at base partition 64 get a base-matched lhsT.
            kbig = spool.tile([128, J3], F32R, tag="kbig")
            nc.vector.tensor_sub(kbig[0:DH, 0:W], ksum_ps[0:DH, :], k7_sb)
            nc.vector.tensor_copy(kbig[0:DH, W:2 * W], ksum_ps[0:DH, :])
            nc.vector.tensor_sub(kbig[0:DH, 2 * W:3 * W], ksum_ps[0:DH, :], k0_sb)
            nc.vector.tensor_copy(kbig[DH:2 * DH, :], kbig[0:DH, :])

            # ---------- Vbig [128, 12, 65(+pad)] ----------
            # chunk j rows p: j-index 128j + p of the 1536; col 64 = ones
            # (softmax denominator accumulator).
            vbig = spool.tile([128, NJ, 68], F32R, tag="vbig")
            ones_sb = spool.tile([128, 1], F32, tag="ones")
            nc.vector.memset(ones_sb, 1.0)
            for j in range(NJ):
                nc.vector.tensor_copy(vbig[:, j, DH:DH + 1], ones_sb)
            for yt in range(4):
                tps = psacc.tile([128, W], F32, tag="acc")
                tp0 = psacc.tile([128, W], F32, tag="acc")
                tp7 = psacc.tile([128, W], F32, tag="acc")
                sl = slice(128 * yt, 128 * (yt + 1))
                nc.tensor.transpose(tps[:, 0:DH], vsum_sb[:, sl], ident[0:DH, 0:DH])
                nc.tensor.transpose(tp0[:, 0:DH], v0_sb[:, sl], ident[0:DH, 0:DH])
                nc.tensor.transpose(tp7[:, 0:DH], v7_sb[:, sl], ident[0:DH, 0:DH])
                # DVE may read only one PSUM operand: evacuate cur first,
                # then subtract the other transposes against the SBUF copy.
                nc.vector.tensor_copy(vbig[:, 4 + yt, 0:DH], tps[:, 0:DH])
                nc.vector.tensor_sub(vbig[:, 0 + yt, 0:DH], vbig[:, 4 + yt, 0:DH], tp7[:, 0:DH])
                nc.vector.tensor_sub(vbig[:, 8 + yt, 0:DH], vbig[:, 4 + yt, 0:DH], tp0[:, 0:DH])

            # ---------- attention (transposed): QK -> exp -> PV ----------
            # denominator rows go through a DRAM scratch because engine APs
            # need 32-aligned base partitions (can't write row h directly).
            dscratch = dpool.tile([NH, W], F32, name="dscratch")
            ctxu_sb = []  # 8 pair tiles [128, 512]: rows 0:64 head 2t, 64:128 head 2t+1
            for t in range(8):
                ctxu_sb.append(qpool.tile([128, W], F32R, tag="qt", name=f"ctxu{t}"))

            for t in range(8):  # head pairs (2t, 2t+1)
                qpt = qpt_sb[t // 2]
                csl = slice(512 * (t % 2), 512 * (t % 2) + W)
                rhsA = qpt[0:DH, csl]
                rhsB = qpt[DH:2 * DH, csl]
                ctxA = psacc.tile([128, W], F32, tag="acc", name=f"ctxA{t}")
                ctxB = psacc.tile([128, W], F32, tag="acc", name=f"ctxB{t}")
                for j in range(NJ):
                    qk = psmm.tile([128, 1024], F32, tag="mm", name=f"qk{t}_{j}")
                    # row-packed pair: even head on PE rows 0:64, odd head on
                    # rows 64:128 (tile_position auto-derived from base
                    # partitions) -> both matmuls run concurrently.
                    nc.tensor.matmul(qk[:, 0:W],
                                     kbig[0:DH, 128 * j:128 * (j + 1)],
                                     rhsA, start=True, stop=True)
                    nc.tensor.matmul(qk[:, W:2 * W],
                                     kbig[DH:2 * DH, 128 * j:128 * (j + 1)],
                                     rhsB, start=True, stop=True)
                    pr = ppool.tile([128, 1024], F32R, tag="probs", name=f"pr{t}_{j}")
                    nc.scalar.activation(pr, qk, AF.Exp, scale=0.125)
                    nc.tensor.matmul(ctxA[0:DH + 1, :], vbig[:, j, 0:DH + 1],
                                     pr[:, 0:W],
                                     start=(j == 0), stop=(j == NJ - 1))
                    nc.tensor.matmul(ctxB[0:DH + 1, :], vbig[:, j, 0:DH + 1],
                                     pr[:, W:2 * W],
                                     start=(j == 0), stop=(j == NJ - 1))
                for h, ctx_ps in ((2 * t, ctxA), (2 * t + 1, ctxB)):
                    dtmp = mpool.tile([1, W], F32, tag="dtmp", name=f"dtmp{h}", bufs=1)
                    nc.vector.tensor_copy(dtmp, ctx_ps[DH:DH + 1, :])
                    nc.sync.dma_start(out=dscratch[h:h + 1, :], in_=dtmp)
                    nc.vector.tensor_copy(
                        ctxu_sb[h // 2][64 * (h % 2):64 * (h % 2) + DH, :],
                        ctx_ps[0:DH, :])
                if t % 2 == 1:
                    # normalize the 2 pairs (4 heads) whose denominators are
                    # complete; earlier batches overlap later pairs' compute.
                    b0 = 4 * (t // 2)
                    dn = mpool.tile([4, W], F32, tag="dn", name=f"dn{t}", bufs=1)
                    nc.scalar.dma_start(out=dn, in_=dscratch[b0:b0 + 4, :])
                    rc = mpool.tile([4, W], F32, tag="rc", name=f"rc{t}", bufs=1)
                    nc.vector.reciprocal(rc, dn)
                    rsc = dpool.tile([4, W], F32, name=f"rsc{t}")
                    nc.scalar.dma_start(out=rsc, in_=rc)
                    for pt in (t - 1, t):
                        bc = mpool.tile([128, W], F32, tag="bcast", name=f"bc{pt}")
                        src = bass.AP(tensor=rsc.tensor,
                                      offset=rsc.offset + (2 * pt - b0) * W,
                                      ap=[[W, 2], [0, DH], [1, W]])
                        nc.scalar.dma_start(out=bc, in_=src)
                        nc.vector.tensor_mul(ctxu_sb[pt], ctxu_sb[pt], bc)

            # ---------- out = ctx @ Wc ----------
            wc_sb = []
            for d in range(DMT):
                t = wpool.tile([128, DM], F32R, tag="wc", name=f"wc{d}")
                nc.gpsimd.dma_start(out=t, in_=Wc[128 * d:128 * (d + 1), :])
                wc_sb.append(t)

            for lt in range(LS // 128):
                ps = psmm.tile([128, 1024], F32, tag="mm")
                for half in range(2):
                    for he in range(DMT):
                        nc.tensor.matmul(
                            ps[:, 512 * half:512 * (half + 1)],
                            ctxu_sb[he][:, 128 * lt:128 * (lt + 1)],
                            wc_sb[he][:, 512 * half:512 * (half + 1)],
                            start=(he == 0), stop=(he == DMT - 1))
                ob = mpool.tile([128, DM], F32, tag="outsb", bufs=1)
                nc.vector.tensor_copy(ob, ps)
                nc.sync.dma_start(out=out[128 * lt:128 * (lt + 1), :], in_=ob)

    nc.compile()
    return nc


_NC = None


def _get_nc():
    global _NC
    if _NC is None:
        _NC = build_nc()
    return _NC


def kernel(q, kv, Wq, Wkv, Wc, w):
    assert int(w) == W
    q = np.asarray(q, dtype=np.float32)
    kv = np.asarray(kv, dtype=np.float32)
    B = q.shape[0]
    assert B == 1 and q.shape[1] == L and q.shape[2] == DM

    qT_full = np.ascontiguousarray(q[0].T)    # [DM, L]
    kvT = np.ascontiguousarray(kv[0].T)       # [DM, L]
    Wq = np.ascontiguousarray(Wq, dtype=np.float32)
    Wkv = np.ascontiguousarray(Wkv, dtype=np.float32)
    Wc = np.ascontiguousarray(Wc, dtype=np.float32)

    in_maps = []
    for i in range(N_CORES):
        in_maps.append({
            "qT": np.ascontiguousarray(qT_full[:, LS * i:LS * (i + 1)]),
            "kvT": kvT,
            "Wq": Wq,
            "Wkv": Wkv,
            "Wc": Wc,
        })

    nc = _get_nc()
    res = run_bass_kernel_spmd(nc, in_maps, list(range(N_CORES)))
    out = np.concatenate([res.results[i]["out"] for i in range(N_CORES)], axis=0)
    return out.reshape(1, L, DM).astype(np.float32)



# revision 7
# speedup vs baseline: 1.1792x; 1.1792x over previous
"""Trainium2 Bass kernel for LocalXLAttention (chunk-summed variant).

Math: the reference einsum sums over the chunk index z, so every query
attends to the same three [w, dh] K/V matrices built from chunk sums:
  K_prev = S_k - k_chunk[C-1], K_cur = S_k, K_next = S_k - k_chunk[0]
(identically for V), where S_k = sum_c k_chunk[c].  Per position l, head h:
  attn[l,h,:]  = qp[l,h,:] @ KbigT          (KbigT: [dh, 3w])
  probs        = softmax(attn, axis=-1)
  ctx[l,h,:]   = probs[l,h,:] @ Vbig        (Vbig:  [3w, dh])
  out          = ctx.reshape(L, dm) @ Wc

Sharding: L=4096 split 512 rows/core across 8 cores (data-parallel over
sequence, no collectives).  Each core redundantly projects the chunk-summed
K/V from the full kv input via PSUM-accumulated matmuls.

All inputs are converted to bf16 on the host (halves DMA, same 1-cycle/row
PE rate as fp32r).  The attention pipeline runs transposed ([dh, l] /
[j, l] layouts); softmax normalization is deferred: an extra all-ones
column of Vbig accumulates the denominator, reciprocals run on DVE
(reciprocal_approx_fast), a K=1 outer-product matmul broadcasts them
across partitions, and a DVE multiply normalizes ctx.

The attention j-loop is software-pipelined by hand (QK of step s+1 is
emitted before PV of step s) so the in-order PE queue never blocks the
Scalar engine's exp stream -- exp is the critical resource (~1.1us per
[128,1024] tile, 96 tiles).
"""

import sys
for _p in ('/opt/pypackages', '/opt/trn_rl_repo'):
    if _p not in sys.path:
        sys.path.insert(0, _p)

import numpy as np
import ml_dtypes

import concourse.bass as bass
import concourse.bacc as bacc
import concourse.tile as tile
from concourse import mybir
from concourse.bass_utils import run_bass_kernel_spmd
from concourse.masks import make_identity

F32 = mybir.dt.float32
F32R = mybir.dt.float32r
BF16 = mybir.dt.bfloat16
AF = mybir.ActivationFunctionType
ALU = mybir.AluOpType

N_CORES = 8
L = 4096          # full sequence
LS = L // N_CORES # 512 rows per core
DM = 1024
NH = 16
DH = 64
W = 512           # chunk width
C = L // W        # 8 chunks
J3 = 3 * W        # 1536 softmax width
NJ = J3 // 128    # 12 j-chunks
DMT = DM // 128   # 8 dm-chunks
NPAIR = 8         # head pairs
NSTEP = NPAIR * NJ


def build_nc():
    nc = bacc.Bacc(None, target_bir_lowering=False)

    qT = nc.dram_tensor("qT", [DM, LS], BF16, kind="ExternalInput")
    kvT = nc.dram_tensor("kvT", [DM, L], BF16, kind="ExternalInput")
    Wq = nc.dram_tensor("Wq", [DM, DM], BF16, kind="ExternalInput")
    Wkv = nc.dram_tensor("Wkv", [DM, 2 * DH], BF16, kind="ExternalInput")
    Wc = nc.dram_tensor("Wc", [DM, DM], BF16, kind="ExternalInput")
    out = nc.dram_tensor("out", [LS, DM], F32, kind="ExternalOutput")

    with tile.TileContext(nc) as tc:
        with tc.tile_pool(name="weights", bufs=8) as wpool, \
             tc.tile_pool(name="small", bufs=1) as spool, \
             tc.tile_pool(name="qp", bufs=8) as qpool, \
             tc.tile_pool(name="qpt", bufs=4) as qptpool, \
             tc.tile_pool(name="stream", bufs=2) as stpool, \
             tc.tile_pool(name="probs", bufs=4) as ppool, \
             tc.tile_pool(name="norm", bufs=4) as npool, \
             tc.tile_pool(name="misc", bufs=2) as mpool, \
             tc.tile_pool(name="psacc", bufs=4, space="PSUM") as psacc, \
             tc.tile_pool(name="psmm", bufs=2, space="PSUM") as psmm:

            # ---------- warm the exp activation table before it matters ----
            dummy = spool.tile([1, 8], F32, tag="dummy")
            nc.vector.memset(dummy, 0.0)
            nc.scalar.activation(dummy, dummy, AF.Exp, scale=0.125)

            # ---------- DMA issues (none on the Scalar engine mid-kernel;
            # scalar only carries prologue kv halves, done before first exp) --
            wkv_sb = []
            for d in range(DMT):
                t = wpool.tile([128, 2 * DH], BF16, tag="wkv", name=f"wkv{d}")
                nc.gpsimd.dma_start(out=t, in_=Wkv[128 * d:128 * (d + 1), :])
                wkv_sb.append(t)
            qt_sb = []
            for d in range(DMT):
                t = qpool.tile([128, LS], BF16, tag="qt", name=f"qt{d}")
                nc.gpsimd.dma_start(out=t, in_=qT[128 * d:128 * (d + 1), :])
                qt_sb.append(t)
            wq_sb = []
            for d in range(DMT):
                t = wpool.tile([128, DM], BF16, tag="wq", name=f"wq{d}")
                nc.gpsimd.dma_start(out=t, in_=Wq[128 * d:128 * (d + 1), :])
                wq_sb.append(t)

            ident = spool.tile([128, 128], F32, tag="ident")
            make_identity(nc, ident)
            ones_sb = spool.tile([1, 128], F32, tag="ones")
            nc.vector.memset(ones_sb, 1.0)

            # ---------- kv stream + PSUM-accumulated projections ----------
            # ps_S = Wkv.T @ (sum_c kv_chunk_c), ps_0/ps_7 = chunk 0/7 proj.
            # rows 0:64 = K, rows 64:128 = V (full-M packed matmuls).
            ps_S = psacc.tile([128, W], F32, tag="acc", name="ps_S")
            ps_0 = psacc.tile([128, W], F32, tag="acc", name="ps_0")
            ps_7 = psacc.tile([128, W], F32, tag="acc", name="ps_7")

            qp_ps = [None] * 4
            qpt_sb = [None] * 4

            def emit_qp_tile(t4, emit_mm_d=None):
                """QP_T tile t4: heads 4t4..4t4+3; accumulate over d."""
                if qp_ps[t4] is None:
                    qp_ps[t4] = psmm.tile([128, 1024], F32, tag="mm",
                                          name=f"qp_ps{t4}")
                ps = qp_ps[t4]
                ds = range(DMT) if emit_mm_d is None else [emit_mm_d]
                for d in ds:
                    for half in range(2):
                        hd = 2 * t4 + half
                        nc.tensor.matmul(
                            ps[:, 512 * half:512 * (half + 1)],
                            wq_sb[d][:, 128 * hd:128 * (hd + 1)],
                            qt_sb[d],
                            start=(d == 0), stop=(d == DMT - 1))

            def finish_qp_tile(t4):
                sb = qptpool.tile([128, 1024], BF16, tag="qpt",
                                  name=f"qpt{t4}")
                nc.vector.tensor_copy(sb, qp_ps[t4])
                qpt_sb[t4] = sb

            # interleave per-d: kv projections chase the kv stream while the
            # first two QP tiles fill (PE executes in order).
            for d in range(DMT):
                st = stpool.tile([128, L], BF16, tag="kvstream", name=f"st{d}")
                eng = nc.sync if d % 2 == 0 else nc.scalar
                eng.dma_start(out=st[:, 0:L // 2],
                              in_=kvT[128 * d:128 * (d + 1), 0:L // 2])
                eng2 = nc.scalar if d % 2 == 0 else nc.sync
                eng2.dma_start(out=st[:, L // 2:L],
                               in_=kvT[128 * d:128 * (d + 1), L // 2:L])
                nc.tensor.matmul(ps_0, wkv_sb[d], st[:, 0:W],
                                 start=(d == 0), stop=(d == DMT - 1))
                nc.tensor.matmul(ps_7, wkv_sb[d], st[:, L - W:L],
                                 start=(d == 0), stop=(d == DMT - 1))
                for c in range(C):
                    nc.tensor.matmul(ps_S, wkv_sb[d], st[:, W * c:W * (c + 1)],
                                     start=(d == 0 and c == 0),
                                     stop=(d == DMT - 1 and c == C - 1))
                # QP tiles 0,1 ride along d-by-d (their weights arrive early)
                emit_qp_tile(0, emit_mm_d=d)
                emit_qp_tile(1, emit_mm_d=d)
            finish_qp_tile(0)
            finish_qp_tile(1)

            # QP tiles 2,3 run while DVE builds kbig below
            emit_qp_tile(2)
            finish_qp_tile(2)
            emit_qp_tile(3)
            finish_qp_tile(3)

            # ---------- Kbig [128, 1536] = [prev | cur | next] (bf16) ------
            s_sb = spool.tile([128, W], F32, tag="ssb")
            nc.vector.tensor_copy(s_sb, ps_S)
            kbig = spool.tile([128, J3], BF16, tag="kbig")
            nc.vector.tensor_sub(kbig[0:DH, 0:W], s_sb[0:DH, :], ps_7[0:DH, :])
            nc.vector.tensor_copy(kbig[0:DH, W:2 * W], s_sb[0:DH, :])
            nc.vector.tensor_sub(kbig[0:DH, 2 * W:3 * W], s_sb[0:DH, :],
                                 ps_0[0:DH, :])
            nc.vector.tensor_copy(kbig[DH:128, :], kbig[0:DH, :])

            # V variants in [dh, l] layout (f32, for PE transpose)
            vprev = spool.tile([DH, W], F32, tag="vprev")
            nc.vector.tensor_sub(vprev, s_sb[DH:128, :], ps_7[DH:128, :])
            vnext = spool.tile([DH, W], F32, tag="vnext")
            nc.vector.tensor_sub(vnext, s_sb[DH:128, :], ps_0[DH:128, :])
            vcur = s_sb[DH:128, :]

            # ---------- Vbig [128, 12, 65(+pad)] bf16 ----------------------
            # row p of chunk j = j-index 128j+p; col 64 = ones (denominator).
            vbig = spool.tile([128, NJ, 68], BF16, tag="vbig")
            ones_col = spool.tile([128, 1], F32, tag="onescol")
            nc.vector.memset(ones_col, 1.0)
            for j in range(NJ):
                nc.vector.tensor_copy(vbig[:, j, DH:DH + 1], ones_col)
            for vi, vsrc in enumerate((vprev, vcur, vnext)):
                # vcur is a slice of s_sb at partition base 64; use the
                # matching diagonal block of the identity so bases agree.
                idsl = ident[DH:128, DH:128] if vi == 1 else ident[0:DH, 0:DH]
                for yt in range(4):
                    tp = psacc.tile([128, W], F32, tag="acc",
                                    name=f"tp{vi}_{yt}")
                    nc.tensor.transpose(tp[:, 0:DH],
                                        vsrc[:, 128 * yt:128 * (yt + 1)],
                                        idsl)
                    nc.vector.tensor_copy(vbig[:, 4 * vi + yt, 0:DH],
                                          tp[:, 0:DH])

            # prefetch Wc during attention
            wc_sb = []
            for d in range(DMT):
                t = wpool.tile([128, DM], BF16, tag="wc", name=f"wc{d}")
                nc.gpsimd.dma_start(out=t, in_=Wc[128 * d:128 * (d + 1), :])
                wc_sb.append(t)

            # ---------- attention: software-pipelined exp-bound loop -------
            ctxu_sb = []  # [128, 512] bf16: rows 0:64 head 2t, 64:128 head 2t+1
            for t in range(NPAIR):
                ctxu_sb.append(qpool.tile([128, W], BF16, tag="qt",
                                          name=f"ctxu{t}"))

            qk_tiles = {}
            ctxA = [None] * NPAIR
            ctxB = [None] * NPAIR

            def emit_qk(s):
                t, j = divmod(s, NJ)
                qk = psmm.tile([128, 1024], F32, tag="mm", name=f"qk{t}_{j}")
                qpt = qpt_sb[t // 2]
                csl = slice(512 * (t % 2), 512 * (t % 2) + W)
                nc.tensor.matmul(qk[:, 0:W],
                                 kbig[0:DH, 128 * j:128 * (j + 1)],
                                 qpt[0:DH, csl], start=True, stop=True)
                nc.tensor.matmul(qk[:, W:2 * W],
                                 kbig[DH:128, 128 * j:128 * (j + 1)],
                                 qpt[DH:128, csl], start=True, stop=True)
                qk_tiles[s] = qk

            def emit_norm(t):
                """normalize pair t: recip(denominator rows) -> K=1 matmul
                broadcast -> ctx copy+mul into ctxu (bf16)."""
                denA = npool.tile([1, W], F32, tag="den", name=f"denA{t}")
                nc.vector.tensor_copy(denA, ctxA[t][DH:DH + 1, :])
                denB = npool.tile([1, W], F32, tag="den", name=f"denB{t}")
                nc.vector.tensor_copy(denB, ctxB[t][DH:DH + 1, :])
                rcpA = npool.tile([1, W], F32, tag="rcp", name=f"rcpA{t}")
                nc.vector.reciprocal_approx_fast(out=rcpA, in_=denA)
                rcpB = npool.tile([1, W], F32, tag="rcp", name=f"rcpB{t}")
                nc.vector.reciprocal_approx_fast(out=rcpB, in_=denB)
                nc.vector.tensor_copy(ctxu_sb[t][0:DH, :], ctxA[t][0:DH, :])
                nc.vector.tensor_copy(ctxu_sb[t][DH:128, :], ctxB[t][0:DH, :])
                bcA = psacc.tile([128, W], F32, tag="acc", name=f"bcA{t}")
                nc.tensor.matmul(bcA, ones_sb, rcpA, start=True, stop=True)
                bcB = psacc.tile([128, W], F32, tag="acc", name=f"bcB{t}")
                nc.tensor.matmul(bcB, ones_sb, rcpB, start=True, stop=True)
                nc.vector.tensor_mul(ctxu_sb[t][0:DH, :],
                                     ctxu_sb[t][0:DH, :], bcA[0:DH, :])
                nc.vector.tensor_mul(ctxu_sb[t][DH:128, :],
                                     ctxu_sb[t][DH:128, :], bcB[0:DH, :])

            emit_qk(0)
            for s in range(NSTEP):
                t, j = divmod(s, NJ)
                qk = qk_tiles.pop(s)
                pr = ppool.tile([128, 1024], BF16, tag="probs",
                                name=f"pr{t}_{j}")
                nc.scalar.activation(pr, qk, AF.Exp, scale=0.125)
                if s + 1 < NSTEP:
                    emit_qk(s + 1)
                if j == 0:
                    ctxA[t] = psacc.tile([128, W], F32, tag="acc",
                                         name=f"ctxA{t}")
                    ctxB[t] = psacc.tile([128, W], F32, tag="acc",
                                         name=f"ctxB{t}")
                nc.tensor.matmul(ctxA[t][0:DH + 1, :], vbig[:, j, 0:DH + 1],
                                 pr[:, 0:W],
                                 start=(j == 0), stop=(j == NJ - 1))
                nc.tensor.matmul(ctxB[t][0:DH + 1, :], vbig[:, j, 0:DH + 1],
                                 pr[:, W:2 * W],
                                 start=(j == 0), stop=(j == NJ - 1))
                # inject previous pair's normalization one step into this
                # pair's j-loop (keeps the PE queue fed while DVE works)
                if j == 1 and t > 0:
                    emit_norm(t - 1)
            emit_norm(NPAIR - 1)

            # ---------- out = ctx @ Wc ----------
            for lt in range(LS // 128):
                ps = psmm.tile([128, 1024], F32, tag="mm", name=f"wo{lt}")
                for half in range(2):
                    for he in range(DMT):
                        nc.tensor.matmul(
                            ps[:, 512 * half:512 * (half + 1)],
                            ctxu_sb[he][:, 128 * lt:128 * (lt + 1)],
                            wc_sb[he][:, 512 * half:512 * (half + 1)],
                            start=(he == 0), stop=(he == DMT - 1))
                ob = mpool.tile([128, DM], F32, tag="outsb", name=f"ob{lt}")
                nc.vector.tensor_copy(ob, ps)
                nc.sync.dma_start(out=out[128 * lt:128 * (lt + 1), :], in_=ob)

    nc.compile()
    return nc


_NC = None


def _get_nc():
    global _NC
    if _NC is None:
        _NC = build_nc()
    return _NC


def prepare_in_maps(q, kv, Wq, Wkv, Wc):
    """Host-side prep: transpose + bf16-cast, shard q over cores."""
    bf = ml_dtypes.bfloat16
    qT_full = np.ascontiguousarray(np.asarray(q, np.float32)[0].T).astype(bf)
    kvT = np.ascontiguousarray(np.asarray(kv, np.float32)[0].T).astype(bf)
    Wq = np.ascontiguousarray(np.asarray(Wq, np.float32)).astype(bf)
    Wkv = np.ascontiguousarray(np.asarray(Wkv, np.float32)).astype(bf)
    Wc = np.ascontiguousarray(np.asarray(Wc, np.float32)).astype(bf)
    in_maps = []
    for i in range(N_CORES):
        in_maps.append({
            "qT": np.ascontiguousarray(qT_full[:, LS * i:LS * (i + 1)]),
            "kvT": kvT,
            "Wq": Wq,
            "Wkv": Wkv,
            "Wc": Wc,
        })
    return in_maps


def kernel(q, kv, Wq, Wkv, Wc, w):
    assert int(w) == W
    q = np.asarray(q, dtype=np.float32)
    assert q.shape[0] == 1 and q.shape[1] == L and q.shape[2] == DM

    in_maps = prepare_in_maps(q, kv, Wq, Wkv, Wc)
    nc = _get_nc()
    res = run_bass_kernel_spmd(nc, in_maps, list(range(N_CORES)))
    out = np.concatenate([res.results[i]["out"] for i in range(N_CORES)],
                         axis=0)
    return out.reshape(1, L, DM).astype(np.float32)


# revision 12
# speedup vs baseline: 1.4658x; 1.2430x over previous
"""Trainium2 Bass kernel for LocalXLAttention (chunk-summed variant).

Math: the reference einsum sums over the chunk index z, so every query
attends to the same three [w, dh] K/V matrices built from chunk sums:
  K_prev = S_k - k_chunk[C-1], K_cur = S_k, K_next = S_k - k_chunk[0]
(identically for V), where S_k = sum_c k_chunk[c].  Per position l, head h:
  attn[l,h,:]  = qp[l,h,:] @ KbigT          (KbigT: [dh, 3w])
  probs        = softmax(attn, axis=-1)
  ctx[l,h,:]   = probs[l,h,:] @ Vbig        (Vbig:  [3w, dh])
  out          = ctx.reshape(L, dm) @ Wc

Sharding: L=4096 split 512 rows/core across 8 cores (data-parallel over
sequence, no collectives).  All inputs are bf16 (host-cast; halves DMA,
same 1-cycle/row PE rate as fp32r).

Schedule notes (engines execute their queues in order, so emission order
IS the schedule):
 - The Scalar engine's exp stream is the critical resource (~1.2us per
   [128,1024] tile, 96 tiles).  The attention loop is software-pipelined:
   QK of step s+1 is emitted right after exp(s); PV runs TWO steps behind
   exp so ctx-psum slot reuse never blocks the PE queue.
 - Prologue PE work is minimized: the kv chunk-sum runs as an in-place
   bf16 tree on the (otherwise idle) DVE, so the S/c0/c7 projections are
   24 matmuls instead of 144; QP head-blocks 0/1 accumulate in the two
   psmm slots during the kv stream; blocks 2/3 are drip-fed into the
   attention loop's PE slack during pairs 0 and 2.
 - Softmax normalization is deferred (an all-ones Vbig column accumulates
   the denominator); per pair: reciprocal_approx_fast on DVE, a
   DRAM-bounce broadcast DMA, and in-place DVE multiplies -- all staged
   across the next pair's steps, nothing on the PE queue.  The last pair
   normalizes via a tiny fp32 K=1 outer-product matmul instead (PE is
   free then and it avoids the DMA round-trip latency in the tail).
"""

import sys
for _p in ('/opt/pypackages', '/opt/trn_rl_repo'):
    if _p not in sys.path:
        sys.path.insert(0, _p)

import numpy as np
import ml_dtypes

import concourse.bass as bass
import concourse.bacc as bacc
import concourse.tile as tile
from concourse import mybir
from concourse.bass_utils import run_bass_kernel_spmd
from concourse.masks import make_identity

F32 = mybir.dt.float32
BF16 = mybir.dt.bfloat16
AF = mybir.ActivationFunctionType
ALU = mybir.AluOpType

N_CORES = 8
L = 4096          # full sequence
LS = L // N_CORES # 512 rows per core
DM = 1024
NH = 16
DH = 64
W = 512           # chunk width
C = L // W        # 8 chunks
J3 = 3 * W        # 1536 softmax width
NJ = J3 // 128    # 12 j-chunks
DMT = DM // 128   # 8 dm-chunks
NPAIR = 8         # head pairs
NSTEP = NPAIR * NJ


def build_nc():
    nc = bacc.Bacc(None, target_bir_lowering=False)

    qT = nc.dram_tensor("qT", [DM, LS], BF16, kind="ExternalInput")
    kvT = nc.dram_tensor("kvT", [DM, L], BF16, kind="ExternalInput")
    Wq = nc.dram_tensor("Wq", [DM, DM], BF16, kind="ExternalInput")
    Wkv = nc.dram_tensor("Wkv", [DM, 2 * DH], BF16, kind="ExternalInput")
    Wc = nc.dram_tensor("Wc", [DM, DM], BF16, kind="ExternalInput")
    out = nc.dram_tensor("out", [LS, DM], F32, kind="ExternalOutput")

    with tile.TileContext(nc) as tc:
        with tc.tile_pool(name="weights", bufs=8) as wpool, \
             tc.tile_pool(name="small", bufs=1) as spool, \
             tc.tile_pool(name="qp", bufs=8) as qpool, \
             tc.tile_pool(name="qpt", bufs=4) as qptpool, \
             tc.tile_pool(name="stream", bufs=4) as stpool, \
             tc.tile_pool(name="ksum", bufs=2) as kspool, \
             tc.tile_pool(name="probs", bufs=6) as ppool, \
             tc.tile_pool(name="norm", bufs=4) as npool, \
             tc.tile_pool(name="bcast", bufs=2) as bcpool, \
             tc.tile_pool(name="misc", bufs=2) as mpool, \
             tc.tile_pool(name="dram", bufs=2, space="DRAM") as dpool, \
             tc.tile_pool(name="psacc", bufs=4, space="PSUM") as psacc, \
             tc.tile_pool(name="psmm", bufs=2, space="PSUM") as psmm:

            # ---------- warm the exp activation table early ----------------
            dummy = spool.tile([1, 8], F32, tag="dummy")
            nc.vector.memset(dummy, 0.0)
            nc.scalar.activation(dummy, dummy, AF.Exp, scale=0.125)

            # ---------- DMA issues (kv on sync+scalar; small stuff spread) -
            wkv_sb = []
            for d in range(DMT):
                t = wpool.tile([128, 2 * DH], BF16, tag="wkv", name=f"wkv{d}")
                nc.gpsimd.dma_start(out=t, in_=Wkv[128 * d:128 * (d + 1), :])
                wkv_sb.append(t)
            qt_sb = []
            for d in range(DMT):
                t = qpool.tile([128, LS], BF16, tag="qt", name=f"qt{d}")
                nc.gpsimd.dma_start(out=t, in_=qT[128 * d:128 * (d + 1), :])
                qt_sb.append(t)
            wq_sb = []
            for d in range(DMT):
                t = wpool.tile([128, DM], BF16, tag="wq", name=f"wq{d}")
                nc.gpsimd.dma_start(out=t, in_=Wq[128 * d:128 * (d + 1), :])
                wq_sb.append(t)

            ident = spool.tile([128, 128], F32, tag="ident")
            make_identity(nc, ident)
            ones_sb = spool.tile([1, 128], F32, tag="ones")
            nc.vector.memset(ones_sb, 1.0)

            # Vbig shell + its ones (denominator) column, built while DMAs run
            vbig = spool.tile([128, NJ, 68], BF16, tag="vbig")
            ones_col = spool.tile([128, 1], F32, tag="onescol")
            nc.vector.memset(ones_col, 1.0)
            for j in range(NJ):
                nc.vector.tensor_copy(vbig[:, j, DH:DH + 1], ones_col)

            # ---------- kv stream -> DVE chunk-sum tree -> projections -----
            # ps_S = Wkv.T @ (sum_c kv_chunk_c); ps_0/ps_7 = chunk 0/7 proj.
            # rows 0:64 = K, rows 64:128 = V (full-M packed matmuls).
            # QP head-blocks 0,1 ([128,1024] psum each) ride along per-d in
            # the two psmm slots.
            ps_S = psacc.tile([128, W], F32, tag="acc", name="ps_S")
            ps_0 = psacc.tile([128, W], F32, tag="acc", name="ps_0")
            ps_7 = psacc.tile([128, W], F32, tag="acc", name="ps_7")
            qp01_ps = [psmm.tile([128, 1024], F32, tag="mm", name=f"qp_ps{g}")
                       for g in range(2)]

            qp_half_ps = {}
            qpt_sb = [None] * 4

            def emit_qp23_half_mms(t4, half, ds):
                """attention-injected QP head-block (t4 in {2,3}): one
                [128, LS] psacc half, accumulated over the given d's."""
                key = (t4, half)
                if key not in qp_half_ps:
                    qp_half_ps[key] = psacc.tile(
                        [128, W], F32, tag="acc", name=f"qph{t4}_{half}")
                ps = qp_half_ps[key]
                hd = 2 * t4 + half
                for d in ds:
                    nc.tensor.matmul(ps, wq_sb[d][:, 128 * hd:128 * (hd + 1)],
                                     qt_sb[d], start=(d == 0),
                                     stop=(d == DMT - 1))

            def emit_qp23_half_copy(t4, half):
                if qpt_sb[t4] is None:
                    qpt_sb[t4] = qptpool.tile([128, 1024], BF16, tag="qpt",
                                              name=f"qpt{t4}")
                nc.vector.tensor_copy(
                    qpt_sb[t4][:, 512 * half:512 * (half + 1)],
                    qp_half_ps.pop((t4, half)))

            # per-d: project c0/c7, QP blocks 0/1, DVE tree-sum, project S
            for d in range(DMT):
                st = stpool.tile([128, L], BF16, tag="kvstream", name=f"st{d}")
                eng = nc.sync if d % 2 == 0 else nc.scalar
                eng.dma_start(out=st[:, 0:L // 2],
                              in_=kvT[128 * d:128 * (d + 1), 0:L // 2])
                eng2 = nc.scalar if d % 2 == 0 else nc.sync
                eng2.dma_start(out=st[:, L // 2:L],
                               in_=kvT[128 * d:128 * (d + 1), L // 2:L])
                nc.tensor.matmul(ps_0, wkv_sb[d], st[:, 0:W],
                                 start=(d == 0), stop=(d == DMT - 1))
                nc.tensor.matmul(ps_7, wkv_sb[d], st[:, L - W:L],
                                 start=(d == 0), stop=(d == DMT - 1))
                for g in range(2):
                    for half in range(2):
                        hd = 2 * g + half
                        nc.tensor.matmul(
                            qp01_ps[g][:, 512 * half:512 * (half + 1)],
                            wq_sb[d][:, 128 * hd:128 * (hd + 1)],
                            qt_sb[d], start=(d == 0), stop=(d == DMT - 1))
                # in-place bf16 tree: chunk sum (c0 slice is read by the ps_0
                # matmul first; c7 slice is never written)
                nc.vector.tensor_add(st[:, 0:2048], st[:, 0:2048],
                                     st[:, 2048:4096])
                nc.vector.tensor_add(st[:, 0:1024], st[:, 0:1024],
                                     st[:, 1024:2048])
                ks = kspool.tile([128, W], BF16, tag="ks", name=f"ks{d}")
                nc.vector.tensor_add(ks, st[:, 0:512], st[:, 512:1024])
                nc.tensor.matmul(ps_S, wkv_sb[d], ks,
                                 start=(d == 0), stop=(d == DMT - 1))
            for g in range(2):
                qpt_sb[g] = qptpool.tile([128, 1024], BF16, tag="qpt",
                                         name=f"qpt{g}")
                nc.vector.tensor_copy(qpt_sb[g], qp01_ps[g])

            # ---------- Kbig [128, 1536] = [prev | cur | next] (bf16) ------
            s_sb = spool.tile([128, W], F32, tag="ssb")
            nc.vector.tensor_copy(s_sb, ps_S)
            kbig = spool.tile([128, J3], BF16, tag="kbig")
            nc.vector.tensor_sub(kbig[0:DH, 0:W], s_sb[0:DH, :], ps_7[0:DH, :])
            nc.vector.tensor_copy(kbig[0:DH, W:2 * W], s_sb[0:DH, :])
            nc.vector.tensor_sub(kbig[0:DH, 2 * W:3 * W], s_sb[0:DH, :],
                                 ps_0[0:DH, :])
            nc.vector.tensor_copy(kbig[DH:128, :], kbig[0:DH, :])

            # V variants in [dh, l] layout (f32, for PE transpose)
            vprev = spool.tile([DH, W], F32, tag="vprev")
            nc.vector.tensor_sub(vprev, s_sb[DH:128, :], ps_7[DH:128, :])
            vnext = spool.tile([DH, W], F32, tag="vnext")
            nc.vector.tensor_sub(vnext, s_sb[DH:128, :], ps_0[DH:128, :])
            vcur = s_sb[DH:128, :]

            # ---------- Vbig payload: 12 PE transposes -> bf16 copies ------
            for vi, vsrc in enumerate((vprev, vcur, vnext)):
                # vcur is a slice of s_sb at partition base 64; use the
                # matching diagonal block of the identity so bases agree.
                idsl = ident[DH:128, DH:128] if vi == 1 else ident[0:DH, 0:DH]
                for yt in range(4):
                    tp = psacc.tile([128, W], F32, tag="acc",
                                    name=f"tp{vi}_{yt}")
                    nc.tensor.transpose(tp[:, 0:DH],
                                        vsrc[:, 128 * yt:128 * (yt + 1)],
                                        idsl)
                    nc.vector.tensor_copy(vbig[:, 4 * vi + yt, 0:DH],
                                          tp[:, 0:DH])

            # ---------- attention: software-pipelined exp-bound loop -------
            ctxu_sb = []  # [128, 512] bf16: rows 0:64 head 2t, 64:128 head 2t+1
            for t in range(NPAIR):
                ctxu_sb.append(qpool.tile([128, W], BF16, tag="ctxu",
                                          name=f"ctxu{t}"))
            wc_sb = [None] * DMT

            qk_tiles = {}
            pr_tiles = {}
            ctxA = [None] * NPAIR
            ctxB = [None] * NPAIR
            norm_state = {}

            def emit_qk(s):
                t, j = divmod(s, NJ)
                qk = psmm.tile([128, 1024], F32, tag="mm", name=f"qk{t}_{j}")
                qpt = qpt_sb[t // 2]
                csl = slice(512 * (t % 2), 512 * (t % 2) + W)
                nc.tensor.matmul(qk[:, 0:W],
                                 kbig[0:DH, 128 * j:128 * (j + 1)],
                                 qpt[0:DH, csl], start=True, stop=True)
                nc.tensor.matmul(qk[:, W:2 * W],
                                 kbig[DH:128, 128 * j:128 * (j + 1)],
                                 qpt[DH:128, csl], start=True, stop=True)
                qk_tiles[s] = qk

            def emit_pv(sv):
                tv, jv = divmod(sv, NJ)
                if jv == 0:
                    ctxA[tv] = psacc.tile([128, W], F32, tag="acc",
                                          name=f"ctxA{tv}")
                    ctxB[tv] = psacc.tile([128, W], F32, tag="acc",
                                          name=f"ctxB{tv}")
                pr = pr_tiles.pop(sv)
                nc.tensor.matmul(ctxA[tv][0:DH + 1, :], vbig[:, jv, 0:DH + 1],
                                 pr[:, 0:W],
                                 start=(jv == 0), stop=(jv == NJ - 1))
                nc.tensor.matmul(ctxB[tv][0:DH + 1, :], vbig[:, jv, 0:DH + 1],
                                 pr[:, W:2 * W],
                                 start=(jv == 0), stop=(jv == NJ - 1))

            def norm_stage(t, stage):
                """staged normalization of pair t (runs during pair t+1)."""
                ns = norm_state.setdefault(t, {})
                if stage == 0:      # denominator rows out of ctx psum
                    ns['denA'] = npool.tile([1, W], F32, tag="den",
                                            name=f"denA{t}")
                    nc.vector.tensor_copy(ns['denA'], ctxA[t][DH:DH + 1, :])
                    ns['denB'] = npool.tile([1, W], F32, tag="den",
                                            name=f"denB{t}")
                    nc.vector.tensor_copy(ns['denB'], ctxB[t][DH:DH + 1, :])
                elif stage == 1:    # reciprocals
                    ns['rcpA'] = npool.tile([1, W], F32, tag="rcp",
                                            name=f"rcpA{t}")
                    nc.vector.reciprocal_approx_fast(out=ns['rcpA'],
                                                     in_=ns['denA'])
                    ns['rcpB'] = npool.tile([1, W], F32, tag="rcp",
                                            name=f"rcpB{t}")
                    nc.vector.reciprocal_approx_fast(out=ns['rcpB'],
                                                     in_=ns['denB'])
                elif stage == 2:    # evacuate ctx psum (releases the slots)
                    nc.vector.tensor_copy(ctxu_sb[t][0:DH, :],
                                          ctxA[t][0:DH, :])
                    nc.vector.tensor_copy(ctxu_sb[t][DH:128, :],
                                          ctxB[t][0:DH, :])
                elif stage == 3:    # reciprocals -> DRAM bounce
                    ns['rsc'] = dpool.tile([2, W], F32, tag="rsc",
                                           name=f"rsc{t}")
                    nc.gpsimd.dma_start(out=ns['rsc'][0:1, :], in_=ns['rcpA'])
                    nc.gpsimd.dma_start(out=ns['rsc'][1:2, :], in_=ns['rcpB'])
                elif stage == 4:    # broadcast-expand back to SBUF
                    ns['bc'] = bcpool.tile([128, W], F32, tag="bc",
                                           name=f"bc{t}")
                    rsc = ns['rsc']
                    src = bass.AP(tensor=rsc.tensor, offset=rsc.offset,
                                  ap=[[W, 2], [0, DH], [1, W]])
                    nc.sync.dma_start(out=ns['bc'], in_=src)
                elif stage == 5:    # in-place normalize, head A
                    nc.vector.tensor_mul(ctxu_sb[t][0:DH, :],
                                         ctxu_sb[t][0:DH, :],
                                         ns['bc'][0:DH, :])
                elif stage == 6:    # in-place normalize, head B
                    nc.vector.tensor_mul(ctxu_sb[t][DH:128, :],
                                         ctxu_sb[t][DH:128, :],
                                         ns['bc'][DH:128, :])

            def emit_norm_tail(t):
                """pair-7 normalization: inline; broadcast via tiny fp32 K=1
                outer-product matmuls (PE is free, no DMA latency)."""
                norm_stage(t, 0)
                norm_stage(t, 1)
                norm_stage(t, 2)
                ns = norm_state[t]
                bcA = psacc.tile([128, W], F32, tag="acc", name=f"bcA{t}")
                nc.tensor.matmul(bcA, ones_sb, ns['rcpA'],
                                 start=True, stop=True)
                bcB = psacc.tile([128, W], F32, tag="acc", name=f"bcB{t}")
                nc.tensor.matmul(bcB, ones_sb, ns['rcpB'],
                                 start=True, stop=True)
                nc.vector.tensor_mul(ctxu_sb[t][0:DH, :],
                                     ctxu_sb[t][0:DH, :], bcA[0:DH, :])
                nc.vector.tensor_mul(ctxu_sb[t][DH:128, :],
                                     ctxu_sb[t][DH:128, :], bcB[0:DH, :])

            # per-exp-step injected work:
            #  - norm(t-1) staged over j==2..10
            #  - QP blocks 2/3 drip-fed during pairs 0/2 at j==5..11 (the
            #    psacc slots are free there: ctx(t-1) was released at j4 and
            #    the qph halves are copied out before ctx(t+1) allocates)
            #  - Wc prefetch issued from the idle gpsimd queue in pair 0
            def injected(t, j):
                if t > 0:
                    stage = {2: 0, 3: 1, 4: 2, 5: 3, 7: 4, 9: 5, 10: 6}.get(j)
                    if stage is not None:
                        norm_stage(t - 1, stage)
                if t in (0, 2):
                    t4 = 2 + t // 2
                    if j in (5, 6, 7, 8):
                        d0 = 2 * (j - 5)
                        emit_qp23_half_mms(t4, 0, [d0, d0 + 1])
                    elif j == 9:
                        emit_qp23_half_mms(t4, 1, [0, 1, 2])
                    elif j == 10:
                        emit_qp23_half_mms(t4, 1, [3, 4, 5])
                    elif j == 11:
                        emit_qp23_half_mms(t4, 1, [6, 7])
                        emit_qp23_half_copy(t4, 0)
                if t in (1, 3) and j == 0:
                    emit_qp23_half_copy(2 + (t - 1) // 2, 1)
                if t == 0 and 2 <= j <= 9:
                    d = j - 2
                    wt = wpool.tile([128, DM], BF16, tag="wc", name=f"wc{d}")
                    nc.gpsimd.dma_start(out=wt,
                                        in_=Wc[128 * d:128 * (d + 1), :])
                    wc_sb[d] = wt

            emit_qk(0)
            for s in range(NSTEP + 2):
                if s < NSTEP:
                    t, j = divmod(s, NJ)
                    qk = qk_tiles.pop(s)
                    pr = ppool.tile([128, 1024], BF16, tag="probs",
                                    name=f"pr{t}_{j}")
                    nc.scalar.activation(pr, qk, AF.Exp, scale=0.125)
                    pr_tiles[s] = pr
                    if s + 1 < NSTEP:
                        emit_qk(s + 1)
                if s >= 2:
                    emit_pv(s - 2)
                if s < NSTEP:
                    injected(t, j)
            emit_norm_tail(NPAIR - 1)

            # ---------- out = ctx @ Wc ----------
            for lt in range(LS // 128):
                ps = psmm.tile([128, 1024], F32, tag="mm", name=f"wo{lt}")
                for half in range(2):
                    for he in range(DMT):
                        nc.tensor.matmul(
                            ps[:, 512 * half:512 * (half + 1)],
                            ctxu_sb[he][:, 128 * lt:128 * (lt + 1)],
                            wc_sb[he][:, 512 * half:512 * (half + 1)],
                            start=(he == 0), stop=(he == DMT - 1))
                ob = mpool.tile([128, DM], F32, tag="outsb", name=f"ob{lt}")
                nc.vector.tensor_copy(ob, ps)
                nc.sync.dma_start(out=out[128 * lt:128 * (lt + 1), :], in_=ob)

    nc.compile()
    return nc


_NC = None


def _get_nc():
    global _NC
    if _NC is None:
        _NC = build_nc()
    return _NC


def prepare_in_maps(q, kv, Wq, Wkv, Wc):
    """Host-side prep: transpose + bf16-cast, shard q over cores."""
    bf = ml_dtypes.bfloat16
    qT_full = np.ascontiguousarray(np.asarray(q, np.float32)[0].T).astype(bf)
    kvT = np.ascontiguousarray(np.asarray(kv, np.float32)[0].T).astype(bf)
    Wq = np.ascontiguousarray(np.asarray(Wq, np.float32)).astype(bf)
    Wkv = np.ascontiguousarray(np.asarray(Wkv, np.float32)).astype(bf)
    Wc = np.ascontiguousarray(np.asarray(Wc, np.float32)).astype(bf)
    in_maps = []
    for i in range(N_CORES):
        in_maps.append({
            "qT": np.ascontiguousarray(qT_full[:, LS * i:LS * (i + 1)]),
            "kvT": kvT,
            "Wq": Wq,
            "Wkv": Wkv,
            "Wc": Wc,
        })
    return in_maps


def kernel(q, kv, Wq, Wkv, Wc, w):
    assert int(w) == W
    q = np.asarray(q, dtype=np.float32)
    assert q.shape[0] == 1 and q.shape[1] == L and q.shape[2] == DM

    in_maps = prepare_in_maps(q, kv, Wq, Wkv, Wc)
    nc = _get_nc()
    res = run_bass_kernel_spmd(nc, in_maps, list(range(N_CORES)))
    out = np.concatenate([res.results[i]["out"] for i in range(N_CORES)],
                         axis=0)
    return out.reshape(1, L, DM).astype(np.float32)
